# revision 1
# baseline (speedup 1.0000x reference)
"""Single-head attention (B=4, N=2048, D=1024) on 8 Trainium2 NeuronCores.

Sharding: core c handles batch c//2 and KEY half c%2.  Each core computes
K/V projections for its 1024 keys, Q for all 2048 queries of its batch, and
partial (unnormalized) attention output plus the partial softmax denominator
over its key half.  The host combines the two halves per batch:
out = (oA + oB) / (dA + dB).  This duplicates only the Q projection across a
core pair (the cheapest of the three), vs duplicating K and V.

All matmuls bf16 with fp32 PSUM accumulation; exp in fp32 on the scalar
engine.  Unnormalized softmax (no max subtraction) is safe: |scores/sqrt(D)|
is ~N(0, 0.33^2) for these inputs.
"""

from contextlib import ExitStack

import ml_dtypes
import numpy as np

import concourse.bass as bass
import concourse.mybir as mybir
import concourse.tile as tile
from concourse.bass_utils import run_bass_kernel_spmd

B, N, D = 4, 2048, 1024
NCORES = 8
P = 128
NQ = N            # queries per core (full batch)
NKH = N // 2      # keys per core (half)
DC = D // P       # 8 contraction chunks
EC = D // P       # 8 embed blocks
JB = NKH // P     # 8 key blocks
F = 512           # matmul moving free dim (one PSUM bank of fp32)
SCALE = 1.0 / np.sqrt(D)

BF = mybir.dt.bfloat16
F32 = mybir.dt.float32


def _attention_kernel(ctx, tc, out, xT, xTk, wqT, wkT, wvT):
    nc = tc.nc

    consts = ctx.enter_context(tc.tile_pool(name="consts", bufs=1))
    psmain = ctx.enter_context(tc.tile_pool(name="psmain", bufs=2, space="PSUM"))
    psav = ctx.enter_context(tc.tile_pool(name="psav", bufs=6, space="PSUM"))
    outp = ctx.enter_context(tc.tile_pool(name="outp", bufs=2))
    small = ctx.enter_context(tc.tile_pool(name="small", bufs=2))

    # Resident SBUF tensors (~170KB/partition).  qT shares the wk+wv buffer:
    # their last reads (phases 1a/1b) precede qT's first write (phase 2a).
    xT_sb = consts.tile([P, DC, NQ], BF, tag="xT")       # [p, d-chunk, query]
    xTk_sb = consts.tile([P, DC, NKH], BF, tag="xTk")    # [p, d-chunk, key]
    wkv_sb = consts.tile([P, 2 * DC * D], BF, tag="wkv")
    wk_sb = wkv_sb.rearrange("p (two c e) -> p two c e", two=2, c=DC)[:, 0]
    wv_sb = wkv_sb.rearrange("p (two c e) -> p two c e", two=2, c=DC)[:, 1]
    qT_sb = wkv_sb.rearrange("p (e i) -> p e i", e=EC)   # [p, e-block, query]
    wq_sb = consts.tile([P, DC, D], BF, tag="wq")
    kT_sb = consts.tile([P, EC, NKH], BF, tag="kT")      # [p, e-block, key]
    v_sb = consts.tile([P, JB, D], BF, tag="v")          # [p, key-block, e]
    pT_sb = consts.tile([P, JB, NQ], BF, tag="pT")       # [p, key-block, query]
    ones_sb = consts.tile([P, 1], BF, tag="ones")

    nc.vector.memset(ones_sb, 1.0)

    xTr = xT.rearrange("(c p) i -> p c i", p=P)
    xTkr = xTk.rearrange("(c p) j -> p c j", p=P)
    wqr = wqT.rearrange("(c p) e -> p c e", p=P)
    wkr = wkT.rearrange("(c p) e -> p c e", p=P)
    wvr = wvT.rearrange("(c p) e -> p c e", p=P)

    # Chunk-0 of every tensor first (the warm-up touch matmuls below gate
    # the PE on exactly these five chunks), then the rest in consumption
    # order (1a: wk+xTk; 1b: wv; 2a: wq+xT).
    in_dmas = []
    wkv_dmas = [None] * (2 * DC)
    wkv_dmas[0] = nc.sync.dma_start(out=wk_sb[:, 0, :], in_=wkr[:, 0, :])
    in_dmas.append(nc.sync.dma_start(out=xTk_sb[:, 0, :], in_=xTkr[:, 0, :]))
    wkv_dmas[DC] = nc.sync.dma_start(out=wv_sb[:, 0, :], in_=wvr[:, 0, :])
    in_dmas.append(nc.sync.dma_start(out=wq_sb[:, 0, :], in_=wqr[:, 0, :]))
    in_dmas.append(nc.sync.dma_start(out=xT_sb[:, 0, :], in_=xTr[:, 0, :]))
    for c in range(1, DC):
        wkv_dmas[c] = nc.sync.dma_start(out=wk_sb[:, c, :], in_=wkr[:, c, :])
        in_dmas.append(nc.sync.dma_start(out=xTk_sb[:, c, :], in_=xTkr[:, c, :]))
    for c in range(1, DC):
        wkv_dmas[DC + c] = nc.sync.dma_start(out=wv_sb[:, c, :], in_=wvr[:, c, :])
    for c in range(1, DC):
        in_dmas.append(nc.sync.dma_start(out=wq_sb[:, c, :], in_=wqr[:, c, :]))
        in_dmas.append(nc.sync.dma_start(out=xT_sb[:, c, :], in_=xTr[:, c, :]))
    in_dmas.extend(wkv_dmas)

    def sp_observe(inst, why):
        # One-wait nops on the sync sequencer: make SP observe a proc's sem
        # tick so later SP instructions (the kernel-tail drain) don't need
        # to aggregate multiple sync waits (HW allows one per instruction).
        n = nc.sync.nop(hint="observe")
        tile.add_dep_helper(n.ins, inst.ins, reason=why)

    # HAM pre-warm: keep the PE busy on dummy matmuls over zeroed SBUF while
    # the first input chunks are still in flight, so the PE clock-gate
    # (which needs ~3.4us of sustained activity) opens before real work.
    warm_src = small.tile([P, 640], BF, tag="warm")
    nc.vector.memset(warm_src, 0.0)
    warm_ps = psmain.tile([P, F], F32, tag="ps")
    N_WARM = 40
    for w in range(N_WARM):
        nc.tensor.matmul(
            warm_ps,
            lhsT=warm_src[:, 0:P],
            rhs=warm_src[:, P : P + F],
            start=(w == 0),
            stop=(w == N_WARM - 1),
        )

    # Warm-up touches: attach each input tensor's chunk-0 DMA wait to a
    # dedicated trivial matmul while PSUM slot reuse is still PE-local, so
    # no later matmul needs a DMA wait on top of a PSUM-WAR wait (PE
    # matmuls support a single sync-wait command).
    for t in (wk_sb, xTk_sb, wv_sb, wq_sb, xT_sb):
        wm = psmain.tile([P, F], F32, tag="ps")
        nc.tensor.matmul(
            wm[0:1, 0:1], lhsT=t[:, 0, 0:1], rhs=t[:, 0, 0:1], start=True, stop=True
        )

    # Phase 1a: kT[e, j] — lhsT = WkT[d, e-blk], rhs = xTk[d, j-tile]
    for e in range(EC):
        for jt in range(NKH // F):
            ps = psmain.tile([P, F], F32, tag="ps")
            for c in range(DC):
                nc.tensor.matmul(
                    ps,
                    lhsT=wk_sb[:, c, e * P : (e + 1) * P],
                    rhs=xTk_sb[:, c, jt * F : (jt + 1) * F],
                    start=(c == 0),
                    stop=(c == DC - 1),
                )
            nc.vector.tensor_copy(out=kT_sb[:, e, jt * F : (jt + 1) * F], in_=ps)

    # Phase 1b: v[j, e] — lhsT = xTk[d, j-blk], rhs = WvT[d, e-tile]
    for j in range(JB):
        for et in range(D // F):
            ps = psmain.tile([P, F], F32, tag="ps")
            for c in range(DC):
                nc.tensor.matmul(
                    ps,
                    lhsT=xTk_sb[:, c, j * P : (j + 1) * P],
                    rhs=wv_sb[:, c, et * F : (et + 1) * F],
                    start=(c == 0),
                    stop=(c == DC - 1),
                )
            nc.vector.tensor_copy(out=v_sb[:, j, et * F : (et + 1) * F], in_=ps)

    # DVE touches: qT overwrites the wk/wv buffer, so the DVE must have
    # observed those input DMAs before its first qT write (WAW), or the qT
    # copies would need a DMA wait on top of their PE wait.  Self-copies
    # carry the DMA waits via explicit deps only.
    touch = small.tile([P, 2 * DC], F32, tag="touch")
    for c in range(2 * DC):
        t = nc.vector.memset(touch[0:1, c : c + 1], 0.0)
        tile.add_dep_helper(t.ins, wkv_dmas[c].ins, reason="observe wkv DMA on DVE")

    # Phase 2a: qT[e, i] for ALL queries of the batch
    for e in range(EC):
        for it in range(NQ // F):
            ps = psmain.tile([P, F], F32, tag="ps")
            for c in range(DC):
                nc.tensor.matmul(
                    ps,
                    lhsT=wq_sb[:, c, e * P : (e + 1) * P],
                    rhs=xT_sb[:, c, it * F : (it + 1) * F],
                    start=(c == 0),
                    stop=(c == DC - 1),
                )
            nc.vector.tensor_copy(out=qT_sb[:, e, it * F : (it + 1) * F], in_=ps)

    # Phase 2b: scoresT[j, i] = k @ q.T over this key half, p = exp(s*SCALE)
    for j in range(JB):
        for it in range(NQ // F):
            ps = psmain.tile([P, F], F32, tag="ps")
            for e in range(EC):
                nc.tensor.matmul(
                    ps,
                    lhsT=kT_sb[:, e, j * P : (j + 1) * P],
                    rhs=qT_sb[:, e, it * F : (it + 1) * F],
                    start=(e == 0),
                    stop=(e == EC - 1),
                )
            last_exp = nc.scalar.activation(
                out=pT_sb[:, j, it * F : (it + 1) * F],
                in_=ps,
                func=mybir.ActivationFunctionType.Exp,
                scale=float(SCALE),
            )

    for d in in_dmas:
        sp_observe(d, "observe input DMA on SP")

    # Phase 2c: partial out[i, 0:1024] = pT.T @ v, partial denom in column
    # 1024 (folded into the same output tensor so there are exactly 8
    # stores — one lap of the 8 SWDGE queues; a second lap would add a
    # queue-order wait on top of the data-ready wait).
    outr = out.rearrange("(g two p) e -> g p two e", two=2, p=P)
    oguard = small.tile([P, NQ // (2 * P)], F32, tag="oguard")
    out_dmas = []
    for ib2 in range(NQ // (2 * P)):
        o_sb = outp.tile([P, 2, D + 1], F32, tag="o")
        g = None
        if ib2 >= 2:
            # Pre-observe the output-DMA tick (WAR on o_sb slot reuse) on
            # the DVE so the copies below carry only their one data wait.
            g = nc.vector.memset(oguard[0:1, ib2 : ib2 + 1], 0.0)
            tile.add_dep_helper(
                g.ins, out_dmas[ib2 - 2].ins, reason="observe out DMA on DVE"
            )
        # Absorb the WAW against the slot's previous DVE writes in a guard
        # write of its own, so the data copies keep a single wait each.
        g2 = nc.vector.memset(o_sb[0:1, 0, 0:1], 0.0)
        if g is not None:
            tile.add_dep_helper(g2.ins, g.ins, False, reason="order after oguard")
        for t in range(2):
            ib = 2 * ib2 + t
            po0 = psav.tile([P, F], F32, tag="po")
            po1 = psav.tile([P, F], F32, tag="po")
            pd = psav.tile([P, F], F32, tag="po")
            for j in range(JB):
                lhsT = pT_sb[:, j, ib * P : (ib + 1) * P]
                nc.tensor.matmul(
                    po0, lhsT=lhsT, rhs=v_sb[:, j, 0:F],
                    start=(j == 0), stop=(j == JB - 1),
                )
                nc.tensor.matmul(
                    po1, lhsT=lhsT, rhs=v_sb[:, j, F : 2 * F],
                    start=(j == 0), stop=(j == JB - 1),
                )
                last_mm = nc.tensor.matmul(
                    pd[:, 0:1], lhsT=lhsT, rhs=ones_sb,
                    start=(j == 0), stop=(j == JB - 1),
                )
            # Denominator copy first: pd's stop-matmul is the group's last
            # PE tick, so this copy's PE wait covers po0/po1 and the po
            # copies need only their (buffer-reuse) DVE wait.  The explicit
            # sync=False deps pin the scheduler to that order.
            dcp = nc.vector.tensor_copy(
                out=o_sb[:, t, D : D + 1], in_=pd[:, 0:1]
            )
            tile.add_dep_helper(dcp.ins, g2.ins, False, reason="order after guard")
            c0 = nc.vector.tensor_copy(out=o_sb[:, t, 0:F], in_=po0)
            tile.add_dep_helper(c0.ins, dcp.ins, False, reason="order after dcp")
            last_cp = nc.vector.tensor_copy(out=o_sb[:, t, F : 2 * F], in_=po1)
            tile.add_dep_helper(last_cp.ins, c0.ins, False, reason="order after c0")
        out_dmas.append(nc.gpsimd.dma_start(out=outr[ib2], in_=o_sb))

    # Let SP observe every remaining proc's final tick so the auto-generated
    # kernel-tail drain needs no aggregated multi-sem wait of its own.
    for dd in out_dmas:
        sp_observe(dd, "observe output DMA on SP")
    sp_observe(last_exp, "observe ACT on SP")
    sp_observe(last_mm, "observe PE on SP")
    sp_observe(last_cp, "observe DVE on SP")


def build_attention_module():
    nc = bass.Bass(trn_type="TRN2", target_bir_lowering=False, debug=False)
    xT = nc.dram_tensor("xT", [D, NQ], BF, kind="ExternalInput").ap()
    xTk = nc.dram_tensor("xTk", [D, NKH], BF, kind="ExternalInput").ap()
    wqT = nc.dram_tensor("wqT", [D, D], BF, kind="ExternalInput").ap()
    wkT = nc.dram_tensor("wkT", [D, D], BF, kind="ExternalInput").ap()
    wvT = nc.dram_tensor("wvT", [D, D], BF, kind="ExternalInput").ap()
    out = nc.dram_tensor("out", [NQ, D + 1], F32, kind="ExternalOutput").ap()
    with tile.TileContext(nc) as tc:
        with ExitStack() as ctx:
            _attention_kernel(ctx, tc, out, xT, xTk, wqT, wkT, wvT)
    return nc


_module_cache = None


def _get_module():
    global _module_cache
    if _module_cache is None:
        _module_cache = build_attention_module()
    return _module_cache


def make_in_maps(x, Wq, Wk, Wv):
    bf = ml_dtypes.bfloat16
    x = np.asarray(x, dtype=np.float32)
    wq = np.asarray(Wq, dtype=np.float32).T.astype(bf)
    wk = np.asarray(Wk, dtype=np.float32).T.astype(bf)
    wv = np.asarray(Wv, dtype=np.float32).T.astype(bf)
    in_maps = []
    for core in range(NCORES):
        b, half = divmod(core, 2)
        xt = np.ascontiguousarray(x[b].T).astype(bf)  # [D, N]
        in_maps.append(
            {
                "xT": xt,
                "xTk": np.ascontiguousarray(xt[:, half * NKH : (half + 1) * NKH]),
                "wqT": wq,
                "wkT": wk,
                "wvT": wv,
            }
        )
    return in_maps


def _install_ntff_hook_shim():
    """The container's `antenv` stub lacks axon_hooks; register an equivalent
    built on trn_agent_boot's ctypes NTFF driver so trace=True works."""
    import sys
    import types

    if "antenv.axon_hooks" in sys.modules:
        return
    try:
        from trn_agent_boot.trn_boot import _ntff_profile_via_ctypes

        hook = _ntff_profile_via_ctypes("/opt/axon/libaxon_pjrt.so")
    except Exception:
        hook = None
    mod = types.ModuleType("antenv.axon_hooks")
    mod.get_axon_ntff_profile_hook = lambda: hook
    sys.modules["antenv.axon_hooks"] = mod


def kernel(x, Wq, Wk, Wv, _trace=False, _trace_cores=None):
    if _trace:
        _install_ntff_hook_shim()
    in_maps = make_in_maps(x, Wq, Wk, Wv)
    nc = _get_module()
    res = run_bass_kernel_spmd(
        nc,
        in_maps,
        core_ids=list(range(NCORES)),
        trace=_trace,
        trace_cores=_trace_cores,
    )
    out = np.empty((B, N, D), dtype=np.float32)
    for b in range(B):
        r0, r1 = res.results[2 * b], res.results[2 * b + 1]
        osum = r0["out"] + r1["out"]
        out[b] = osum[:, :D] / osum[:, D : D + 1]
    if _trace:
        return out, res
    return out



# revision 2
# speedup vs baseline: 1.0059x; 1.0059x over previous
"""Single-head attention (B=4, N=2048, D=1024) on 8 Trainium2 NeuronCores.

Sharding: core c handles batch c//2 and KEY half c%2.  The host permutes
each core's xT so its key half occupies query columns 0..1023; the kernel
then slices keys out of xT directly (no separate xTk input).  Each core
computes K/V projections for its 1024 keys, Q for all 2048 (permuted)
queries, and partial (unnormalized) attention output plus the partial
softmax denominator over its key half, in bf16.  The host un-permutes the
odd cores' query rows and combines the two halves per batch:
out = (oA + oB) / (dA + dB).

Input DMA: xT key-half chunks stream on the sync HWDGE queues while wk
chunks stream on the scalar HWDGE queues in parallel; wv / wq / xT
query-half follow as single large DMAs (their consumers run much later).
Phase 1a's first e-block is chunk-gated and interleaved with touch
matmuls so the PE has paced work (and HAM warms) while inputs stream.

All matmuls bf16 with fp32 PSUM accumulation; exp in fp32 on the scalar
engine.  Unnormalized softmax (no max subtraction) is safe: |scores/sqrt(D)|
is ~N(0, 0.33^2) for these inputs.  Partial outputs are stored bf16.
"""

from contextlib import ExitStack

import ml_dtypes
import numpy as np

import concourse.bass as bass
import concourse.mybir as mybir
import concourse.tile as tile
from concourse.bass_utils import run_bass_kernel_spmd

B, N, D = 4, 2048, 1024
NCORES = 8
P = 128
NQ = N            # queries per core (full batch, permuted: key half first)
NKH = N // 2      # keys per core (half)
DC = D // P       # 8 contraction chunks
EC = D // P       # 8 embed blocks
JB = NKH // P     # 8 key blocks
F = 512           # matmul moving free dim (one PSUM bank of fp32)
SCALE = 1.0 / np.sqrt(D)
N_WARM_PRE = 6    # dummy matmuls before the first chunk-gated matmul

BF = mybir.dt.bfloat16
F32 = mybir.dt.float32


def _attention_kernel(ctx, tc, out, xT, wqT, wkT, wvT):
    nc = tc.nc

    consts = ctx.enter_context(tc.tile_pool(name="consts", bufs=1))
    psmain = ctx.enter_context(tc.tile_pool(name="psmain", bufs=2, space="PSUM"))
    psav = ctx.enter_context(tc.tile_pool(name="psav", bufs=6, space="PSUM"))
    outp = ctx.enter_context(tc.tile_pool(name="outp", bufs=2))
    small = ctx.enter_context(tc.tile_pool(name="small", bufs=2))

    # Resident SBUF tensors.  qT shares the wk+wv buffer: their last reads
    # (phases 1a/1b) precede qT's first write (phase 2a).
    xT_sb = consts.tile([P, DC, NQ], BF, tag="xT")       # [p, d-chunk, query]
    wkv_sb = consts.tile([P, 2 * DC * D], BF, tag="wkv")
    wk_sb = wkv_sb.rearrange("p (two c e) -> p two c e", two=2, c=DC)[:, 0]
    wv_sb = wkv_sb.rearrange("p (two c e) -> p two c e", two=2, c=DC)[:, 1]
    qT_sb = wkv_sb.rearrange("p (e i) -> p e i", e=EC)   # [p, e-block, query]
    wq_sb = consts.tile([P, DC, D], BF, tag="wq")
    kT_sb = consts.tile([P, EC, NKH], BF, tag="kT")      # [p, e-block, key]
    v_sb = consts.tile([P, JB, D], BF, tag="v")          # [p, key-block, e]
    pT_sb = consts.tile([P, JB, NQ], BF, tag="pT")       # [p, key-block, query]
    ones_sb = consts.tile([P, 1], BF, tag="ones")

    nc.vector.memset(ones_sb, 1.0)

    xTr = xT.rearrange("(c p) i -> p c i", p=P)
    wqr = wqT.rearrange("(c p) e -> p c e", p=P)
    wkr = wkT.rearrange("(c p) e -> p c e", p=P)
    wvr = wvT.rearrange("(c p) e -> p c e", p=P)

    # Input DMAs.  Phase 1a needs wk + xT[:, 0:NKH] chunk-by-chunk ASAP:
    # xk chunks trigger on the sync HWDGE queues, wk chunks on the scalar
    # HWDGE queues (parallel trigger issue).  wv / wq / xT-query-half are
    # needed only by phases 1b (t+28us) and 2a (t+75us): single large DMAs
    # triggered after the chunked ones.
    xk_dmas = []
    wk_dmas = []
    for c in range(DC):
        xk_dmas.append(
            nc.sync.dma_start(out=xT_sb[:, c, 0:NKH], in_=xTr[:, c, 0:NKH])
        )
        wk_dmas.append(nc.scalar.dma_start(out=wk_sb[:, c, :], in_=wkr[:, c, :]))
    wv_dma = nc.scalar.dma_start(out=wv_sb[:, :, :], in_=wvr)
    wq_dma = nc.sync.dma_start(out=wq_sb[:, :, :], in_=wqr)
    xq_dma = nc.sync.dma_start(out=xT_sb[:, :, NKH:NQ], in_=xTr[:, :, NKH:NQ])
    in_dmas = xk_dmas + wk_dmas + [wv_dma, wq_dma, xq_dma]

    def sp_observe(inst, why):
        # One-wait nops on the sync sequencer: make SP observe a proc's sem
        # tick so later SP instructions (the kernel-tail drain) don't need
        # to aggregate multiple sync waits (HW allows one per instruction).
        n = nc.sync.nop(hint="observe")
        tile.add_dep_helper(n.ins, inst.ins, reason=why)

    # One PSUM tile for dummy warm-up and touch matmuls.  It comes from the
    # psav pool, which no DVE copy reads until phase 2c — so every write to
    # it is PE-local and touch matmuls carry exactly one (DMA) wait.
    warm_src = small.tile([P, 640], BF, tag="warm")
    nc.vector.memset(warm_src, 0.0)
    warm_ps = psav.tile([P, F], F32, tag="po")

    def dummy():
        nc.tensor.matmul(
            warm_ps, lhsT=warm_src[:, 0:P], rhs=warm_src[:, P : P + F],
            start=True, stop=True,
        )

    def touch(t):
        # Trivial matmul whose only purpose is to make the PE observe t's
        # DMA (single sync wait), so later real matmuls reading t need none.
        nc.tensor.matmul(
            warm_ps[0:1, 0:1], lhsT=t[:, 0:1], rhs=t[:, 0:1], start=True, stop=True
        )

    for _ in range(N_WARM_PRE):
        dummy()

    # Phase 1a: kT[e, j] — lhsT = WkT[d, e-blk], rhs = xT[d, j-tile] (keys).
    # e=0 runs chunk-major, gated on each (xk, wk) chunk pair as it lands,
    # with the touch matmuls carrying the DMA waits and keeping the PE paced.
    ps0 = psmain.tile([P, F], F32, tag="ps")
    ps1 = psmain.tile([P, F], F32, tag="ps")
    for c in range(DC):
        touch(xT_sb[:, c, 0:NKH])
        touch(wk_sb[:, c, :])
        nc.tensor.matmul(
            ps0, lhsT=wk_sb[:, c, 0:P], rhs=xT_sb[:, c, 0:F],
            start=(c == 0), stop=(c == DC - 1),
        )
        nc.tensor.matmul(
            ps1, lhsT=wk_sb[:, c, 0:P], rhs=xT_sb[:, c, F : 2 * F],
            start=(c == 0), stop=(c == DC - 1),
        )
    nc.vector.tensor_copy(out=kT_sb[:, 0, 0:F], in_=ps0)
    nc.vector.tensor_copy(out=kT_sb[:, 0, F : 2 * F], in_=ps1)

    for e in range(1, EC):
        for jt in range(NKH // F):
            ps = psmain.tile([P, F], F32, tag="ps")
            for c in range(DC):
                nc.tensor.matmul(
                    ps,
                    lhsT=wk_sb[:, c, e * P : (e + 1) * P],
                    rhs=xT_sb[:, c, jt * F : (jt + 1) * F],
                    start=(c == 0),
                    stop=(c == DC - 1),
                )
            nc.vector.tensor_copy(out=kT_sb[:, e, jt * F : (jt + 1) * F], in_=ps)
        if e == 4:
            # wv lands ~17-19us; PE reaches this point ~22us — absorb the
            # wv DMA wait here so phase 1b matmuls need none.
            touch(wv_sb[:, 0, :])

    # Phase 1b: v[j, e] — lhsT = xT[d, j-blk] (keys), rhs = WvT[d, e-tile]
    for j in range(JB):
        for et in range(D // F):
            ps = psmain.tile([P, F], F32, tag="ps")
            for c in range(DC):
                nc.tensor.matmul(
                    ps,
                    lhsT=xT_sb[:, c, j * P : (j + 1) * P],
                    rhs=wv_sb[:, c, et * F : (et + 1) * F],
                    start=(c == 0),
                    stop=(c == DC - 1),
                )
            nc.vector.tensor_copy(out=v_sb[:, j, et * F : (et + 1) * F], in_=ps)
        if j == 1:
            # wq / xT-query-half land ~20-25us; PE reaches this ~52us.
            touch(wq_sb[:, 0, :])
            touch(xT_sb[:, 0, NKH : NKH + P])

    # DVE touches: qT overwrites the wk/wv buffer, so the DVE must have
    # observed those input DMAs before its first qT write (WAW), or the qT
    # copies would need a DMA wait on top of their PE wait.
    touch_sb = small.tile([P, DC + 1], F32, tag="touch")
    for i, dma in enumerate(wk_dmas + [wv_dma]):
        t = nc.vector.memset(touch_sb[0:1, i : i + 1], 0.0)
        tile.add_dep_helper(t.ins, dma.ins, reason="observe wkv DMA on DVE")

    # Phase 2a: qT[e, i] for ALL (permuted) queries of the batch
    for e in range(EC):
        for it in range(NQ // F):
            ps = psmain.tile([P, F], F32, tag="ps")
            for c in range(DC):
                nc.tensor.matmul(
                    ps,
                    lhsT=wq_sb[:, c, e * P : (e + 1) * P],
                    rhs=xT_sb[:, c, it * F : (it + 1) * F],
                    start=(c == 0),
                    stop=(c == DC - 1),
                )
            nc.vector.tensor_copy(out=qT_sb[:, e, it * F : (it + 1) * F], in_=ps)

    # Phase 2b: scoresT[j, i] = k @ q.T over this key half, p = exp(s*SCALE)
    for j in range(JB):
        for it in range(NQ // F):
            ps = psmain.tile([P, F], F32, tag="ps")
            for e in range(EC):
                nc.tensor.matmul(
                    ps,
                    lhsT=kT_sb[:, e, j * P : (j + 1) * P],
                    rhs=qT_sb[:, e, it * F : (it + 1) * F],
                    start=(e == 0),
                    stop=(e == EC - 1),
                )
            last_exp = nc.scalar.activation(
                out=pT_sb[:, j, it * F : (it + 1) * F],
                in_=ps,
                func=mybir.ActivationFunctionType.Exp,
                scale=float(SCALE),
            )

    for d in in_dmas:
        sp_observe(d, "observe input DMA on SP")

    # Phase 2c: partial out[i, 0:1024] = pT.T @ v, partial denom in column
    # 1024 (folded into the same output tensor so there are exactly 8
    # stores — one lap of the 8 SWDGE queues; a second lap would add a
    # queue-order wait on top of the data-ready wait).  Stored bf16.
    outr = out.rearrange("(g two p) e -> g p two e", two=2, p=P)
    oguard = small.tile([P, NQ // (2 * P)], F32, tag="oguard")
    out_dmas = []
    for ib2 in range(NQ // (2 * P)):
        o_sb = outp.tile([P, 2, D + 1], BF, tag="o")
        g = None
        if ib2 >= 2:
            # Pre-observe the output-DMA tick (WAR on o_sb slot reuse) on
            # the DVE so the copies below carry only their one data wait.
            g = nc.vector.memset(oguard[0:1, ib2 : ib2 + 1], 0.0)
            tile.add_dep_helper(
                g.ins, out_dmas[ib2 - 2].ins, reason="observe out DMA on DVE"
            )
        # Absorb the WAW against the slot's previous DVE writes in a guard
        # write of its own, so the data copies keep a single wait each.
        g2 = nc.vector.memset(o_sb[0:1, 0, 0:1], 0.0)
        if g is not None:
            tile.add_dep_helper(g2.ins, g.ins, False, reason="order after oguard")
        for t in range(2):
            ib = 2 * ib2 + t
            po0 = psav.tile([P, F], F32, tag="po")
            po1 = psav.tile([P, F], F32, tag="po")
            pd = psav.tile([P, F], F32, tag="po")
            for j in range(JB):
                lhsT = pT_sb[:, j, ib * P : (ib + 1) * P]
                nc.tensor.matmul(
                    po0, lhsT=lhsT, rhs=v_sb[:, j, 0:F],
                    start=(j == 0), stop=(j == JB - 1),
                )
                nc.tensor.matmul(
                    po1, lhsT=lhsT, rhs=v_sb[:, j, F : 2 * F],
                    start=(j == 0), stop=(j == JB - 1),
                )
                last_mm = nc.tensor.matmul(
                    pd[:, 0:1], lhsT=lhsT, rhs=ones_sb,
                    start=(j == 0), stop=(j == JB - 1),
                )
            # Denominator copy first: pd's stop-matmul is the group's last
            # PE tick, so this copy's PE wait covers po0/po1 and the po
            # copies need only their (buffer-reuse) DVE wait.  The explicit
            # sync=False deps pin the scheduler to that order.
            dcp = nc.vector.tensor_copy(
                out=o_sb[:, t, D : D + 1], in_=pd[:, 0:1]
            )
            tile.add_dep_helper(dcp.ins, g2.ins, False, reason="order after guard")
            c0 = nc.vector.tensor_copy(out=o_sb[:, t, 0:F], in_=po0)
            tile.add_dep_helper(c0.ins, dcp.ins, False, reason="order after dcp")
            last_cp = nc.vector.tensor_copy(out=o_sb[:, t, F : 2 * F], in_=po1)
            tile.add_dep_helper(last_cp.ins, c0.ins, False, reason="order after c0")
        out_dmas.append(nc.gpsimd.dma_start(out=outr[ib2], in_=o_sb))

    # Let SP observe every remaining proc's final tick so the auto-generated
    # kernel-tail drain needs no aggregated multi-sem wait of its own.
    for dd in out_dmas:
        sp_observe(dd, "observe output DMA on SP")
    sp_observe(last_exp, "observe ACT on SP")
    sp_observe(last_mm, "observe PE on SP")
    sp_observe(last_cp, "observe DVE on SP")


def build_attention_module():
    nc = bass.Bass(trn_type="TRN2", target_bir_lowering=False, debug=False)
    xT = nc.dram_tensor("xT", [D, NQ], BF, kind="ExternalInput").ap()
    wqT = nc.dram_tensor("wqT", [D, D], BF, kind="ExternalInput").ap()
    wkT = nc.dram_tensor("wkT", [D, D], BF, kind="ExternalInput").ap()
    wvT = nc.dram_tensor("wvT", [D, D], BF, kind="ExternalInput").ap()
    out = nc.dram_tensor("out", [NQ, D + 1], BF, kind="ExternalOutput").ap()
    with tile.TileContext(nc) as tc:
        with ExitStack() as ctx:
            _attention_kernel(ctx, tc, out, xT, wqT, wkT, wvT)
    return nc


_module_cache = None


def _get_module():
    global _module_cache
    if _module_cache is None:
        _module_cache = build_attention_module()
    return _module_cache


def make_in_maps(x, Wq, Wk, Wv):
    bf = ml_dtypes.bfloat16
    x = np.asarray(x, dtype=np.float32)
    wq = np.asarray(Wq, dtype=np.float32).T.astype(bf)
    wk = np.asarray(Wk, dtype=np.float32).T.astype(bf)
    wv = np.asarray(Wv, dtype=np.float32).T.astype(bf)
    in_maps = []
    for core in range(NCORES):
        b, half = divmod(core, 2)
        xt = x[b].T  # [D, N]
        if half:
            xt = np.concatenate([xt[:, NKH:], xt[:, :NKH]], axis=1)
        in_maps.append(
            {
                "xT": np.ascontiguousarray(xt).astype(bf),
                "wqT": wq,
                "wkT": wk,
                "wvT": wv,
            }
        )
    return in_maps


def _install_ntff_hook_shim():
    """The container's `antenv` stub lacks axon_hooks; register an equivalent
    built on trn_agent_boot's ctypes NTFF driver so trace=True works."""
    import sys
    import types

    if "antenv.axon_hooks" in sys.modules:
        return
    try:
        from trn_agent_boot.trn_boot import _ntff_profile_via_ctypes

        hook = _ntff_profile_via_ctypes("/opt/axon/libaxon_pjrt.so")
    except Exception:
        hook = None
    mod = types.ModuleType("antenv.axon_hooks")
    mod.get_axon_ntff_profile_hook = lambda: hook
    sys.modules["antenv.axon_hooks"] = mod


def kernel(x, Wq, Wk, Wv, _trace=False, _trace_cores=None):
    if _trace:
        _install_ntff_hook_shim()
    in_maps = make_in_maps(x, Wq, Wk, Wv)
    nc = _get_module()
    res = run_bass_kernel_spmd(
        nc,
        in_maps,
        core_ids=list(range(NCORES)),
        trace=_trace,
        trace_cores=_trace_cores,
    )
    out = np.empty((B, N, D), dtype=np.float32)
    for b in range(B):
        r0 = res.results[2 * b]["out"].astype(np.float32)
        r1 = res.results[2 * b + 1]["out"].astype(np.float32)
        # core 2b+1's query rows are permuted (its key half first): un-permute.
        r1 = np.concatenate([r1[NKH:], r1[:NKH]], axis=0)
        osum = r0 + r1
        out[b] = osum[:, :D] / osum[:, D : D + 1]
    if _trace:
        return out, res
    return out


# revision 7
# speedup vs baseline: 1.0283x; 1.0222x over previous
"""Single-head attention (B=4, N=2048, D=1024) on 8 Trainium2 NeuronCores.

Sharding: core c handles batch c//2 and KEY half c%2.  The host permutes
each core's xT so its key half occupies query columns 0..1023; the kernel
then slices keys out of xT directly (no separate xTk input).  Each core
computes K/V projections for its 1024 keys, Q for all 2048 (permuted)
queries, and partial (unnormalized) attention output plus the partial
softmax denominator over its key half, in bf16.  The host un-permutes the
odd cores' query rows and combines the two halves per batch:
out = (oA + oB) / (dA + dB).

Input DMA: xT key-half chunks stream on the sync HWDGE queues while wk
chunks stream on the scalar HWDGE queues in parallel; wv / wq / xT
query-half follow as single large DMAs (their consumers run much later).
Phase 1a's first e-block is chunk-gated and interleaved with touch
matmuls so the PE has paced work (and HAM warms) while inputs stream.

All matmuls bf16 with fp32 PSUM accumulation; exp in fp32 on the scalar
engine.  Unnormalized softmax (no max subtraction) is safe: |scores/sqrt(D)|
is ~N(0, 0.33^2) for these inputs.  Partial outputs are stored bf16.
"""

from contextlib import ExitStack

import ml_dtypes
import numpy as np

import concourse.bass as bass
import concourse.mybir as mybir
import concourse.tile as tile
from concourse.bass_utils import run_bass_kernel_spmd

B, N, D = 4, 2048, 1024
NCORES = 8
P = 128
NQ = N            # queries per core (full batch, permuted: key half first)
NKH = N // 2      # keys per core (half)
DC = D // P       # 8 contraction chunks
EC = D // P       # 8 embed blocks
JB = NKH // P     # 8 key blocks
F = 512           # matmul moving free dim (one PSUM bank of fp32)
SCALE = 1.0 / np.sqrt(D)
N_WARM_PRE = 9    # back-to-back dummy matmuls (~3.8us cold) to warm HAM

BF = mybir.dt.bfloat16
F32 = mybir.dt.float32


def _attention_kernel(ctx, tc, out, xT, wqT, wkT, wvT):
    nc = tc.nc

    consts = ctx.enter_context(tc.tile_pool(name="consts", bufs=1))
    psmain = ctx.enter_context(tc.tile_pool(name="psmain", bufs=2, space="PSUM"))
    psav = ctx.enter_context(tc.tile_pool(name="psav", bufs=6, space="PSUM"))
    outp = ctx.enter_context(tc.tile_pool(name="outp", bufs=2))
    small = ctx.enter_context(tc.tile_pool(name="small", bufs=2))

    # Resident SBUF tensors.  qT shares the wk+wv buffer: their last reads
    # (phases 1a/1b) precede qT's first write (phase 2a).
    xT_sb = consts.tile([P, DC, NQ], BF, tag="xT")       # [p, d-chunk, query]
    wkv_sb = consts.tile([P, 2 * DC * D], BF, tag="wkv")
    wk_sb = wkv_sb.rearrange("p (two c e) -> p two c e", two=2, c=DC)[:, 0]
    wv_sb = wkv_sb.rearrange("p (two c e) -> p two c e", two=2, c=DC)[:, 1]
    qT_sb = wkv_sb.rearrange("p (e i) -> p e i", e=EC)   # [p, e-block, query]
    wq_sb = consts.tile([P, DC, D], BF, tag="wq")
    kT_sb = consts.tile([P, EC, NKH], BF, tag="kT")      # [p, e-block, key]
    v_sb = consts.tile([P, JB, D], BF, tag="v")          # [p, key-block, e]
    pT_sb = consts.tile([P, JB, NQ], BF, tag="pT")       # [p, key-block, query]
    ones_sb = consts.tile([P, 1], BF, tag="ones")

    nc.vector.memset(ones_sb, 1.0)

    xTr = xT.rearrange("(c p) i -> p c i", p=P)
    wqr = wqT.rearrange("(c p) e -> p c e", p=P)
    wkr = wkT.rearrange("(c p) e -> p c e", p=P)
    wvr = wvT.rearrange("(c p) e -> p c e", p=P)

    # Input DMAs.  The per-core HBM read port (~358 GB/s) is the early
    # bottleneck, not trigger issue: 10MB of input takes ~28us to land.
    # Phase 1a needs wk + xT[:, 0:NKH] (4MB) chunk-by-chunk ASAP; wv / wq /
    # xT-query-half are needed only by phases 1b (t+45us) and 2a (t+72us).
    # Chain the big DMAs' triggers behind the previous tensor's completion
    # so the 16 round-robin HWDGE queues don't steal read bandwidth from
    # the critical wk+xk stream.
    xk_dmas = []
    wk_dmas = []
    for c in range(DC):
        xk_dmas.append(
            nc.sync.dma_start(out=xT_sb[:, c, 0:NKH], in_=xTr[:, c, 0:NKH])
        )
        wk_dmas.append(nc.sync.dma_start(out=wk_sb[:, c, :], in_=wkr[:, c, :]))
    # The gate wait lives on an SP nop (DMA triggers allow at most one sync
    # wait and may already carry a queue-lap wait); the trigger follows the
    # nop in SP program order (sync=False edge).
    def chained_dma(gate, out_ap, in_ap):
        n = nc.sync.nop(hint="chain")
        tile.add_dep_helper(n.ins, gate.ins, reason="DMA priority chain")
        d = nc.sync.dma_start(out=out_ap, in_=in_ap)
        tile.add_dep_helper(d.ins, n.ins, False, reason="order after chain nop")
        return d

    wv_dma = chained_dma(wk_dmas[-1], wv_sb[:, :, :], wvr)
    wq_dma = chained_dma(wv_dma, wq_sb[:, :, :], wqr)
    xq_dma = chained_dma(wq_dma, xT_sb[:, :, NKH:NQ], xTr[:, :, NKH:NQ])
    in_dmas = xk_dmas + wk_dmas + [wv_dma, wq_dma, xq_dma]

    def sp_observe(inst, why):
        # One-wait nops on the sync sequencer: make SP observe a proc's sem
        # tick so later SP instructions (the kernel-tail drain) don't need
        # to aggregate multiple sync waits (HW allows one per instruction).
        n = nc.sync.nop(hint="observe")
        tile.add_dep_helper(n.ins, inst.ins, reason=why)

    # One PSUM tile for dummy warm-up and touch matmuls.  It comes from the
    # psav pool, which no DVE copy reads until phase 2c — so every write to
    # it is PE-local and touch matmuls carry exactly one (DMA) wait.
    warm_src = small.tile([P, 640], BF, tag="warm")
    nc.vector.memset(warm_src, 0.0)
    warm_ps = psav.tile([P, F], F32, tag="po")

    def dummy():
        nc.tensor.matmul(
            warm_ps, lhsT=warm_src[:, 0:P], rhs=warm_src[:, P : P + F],
            start=True, stop=True,
        )

    def touch(t):
        # Trivial matmul whose only purpose is to make the PE observe t's
        # DMA (single sync wait), so later real matmuls reading t need none.
        nc.tensor.matmul(
            warm_ps[0:1, 0:1], lhsT=t[:, 0:1], rhs=t[:, 0:1], start=True, stop=True
        )

    # Solid warm-up block: HAM un-throttles only after a ~3.4us window of
    # SUSTAINED PE activity; scattered chunk-gated matmuls never produce
    # one, so pay for it up front with back-to-back dummies (~3.8us cold).
    for _ in range(N_WARM_PRE):
        dummy()

    # Phase 1a: kT[e, j] — lhsT = WkT[d, e-blk], rhs = xT[d, j-tile] (keys).
    # e=0 runs chunk-major, gated on each (xk, wk) chunk pair as it lands
    # (~1.4us apart at HBM rate), with the touch matmuls carrying the DMA
    # waits and interleaved dummies keeping the PE duty high so HAM stays
    # warm.
    ps0 = psmain.tile([P, F], F32, tag="ps")
    ps1 = psmain.tile([P, F], F32, tag="ps")
    for c in range(DC):
        touch(xT_sb[:, c, 0:NKH])
        touch(wk_sb[:, c, :])
        nc.tensor.matmul(
            ps0, lhsT=wk_sb[:, c, 0:P], rhs=xT_sb[:, c, 0:F],
            start=(c == 0), stop=(c == DC - 1),
        )
        nc.tensor.matmul(
            ps1, lhsT=wk_sb[:, c, 0:P], rhs=xT_sb[:, c, F : 2 * F],
            start=(c == 0), stop=(c == DC - 1),
        )
        dummy()
        dummy()
    nc.vector.tensor_copy(out=kT_sb[:, 0, 0:F], in_=ps0)
    nc.vector.tensor_copy(out=kT_sb[:, 0, F : 2 * F], in_=ps1)

    for e in range(1, EC):
        for jt in range(NKH // F):
            ps = psmain.tile([P, F], F32, tag="ps")
            for c in range(DC):
                nc.tensor.matmul(
                    ps,
                    lhsT=wk_sb[:, c, e * P : (e + 1) * P],
                    rhs=xT_sb[:, c, jt * F : (jt + 1) * F],
                    start=(c == 0),
                    stop=(c == DC - 1),
                )
            nc.vector.tensor_copy(out=kT_sb[:, e, jt * F : (jt + 1) * F], in_=ps)
        if e == 6:
            # wv lands ~25us (chained behind the 4MB wk+xk stream); the PE
            # reaches this point ~40us — absorb the wv DMA wait here so
            # phase 1b matmuls need none.
            touch(wv_sb[:, 0, :])

    # Phase 1b: v[j, e] — lhsT = xT[d, j-blk] (keys), rhs = WvT[d, e-tile]
    for j in range(JB):
        for et in range(D // F):
            ps = psmain.tile([P, F], F32, tag="ps")
            for c in range(DC):
                nc.tensor.matmul(
                    ps,
                    lhsT=xT_sb[:, c, j * P : (j + 1) * P],
                    rhs=wv_sb[:, c, et * F : (et + 1) * F],
                    start=(c == 0),
                    stop=(c == DC - 1),
                )
            nc.vector.tensor_copy(out=v_sb[:, j, et * F : (et + 1) * F], in_=ps)
        if j == 1:
            # wq / xT-query-half land ~20-25us; PE reaches this ~52us.
            touch(wq_sb[:, 0, :])
            touch(xT_sb[:, 0, NKH : NKH + P])

    # DVE touches: qT overwrites the wk/wv buffer, so the DVE must have
    # observed those input DMAs before its first qT write (WAW), or the qT
    # copies would need a DMA wait on top of their PE wait.
    touch_sb = small.tile([P, DC + 1], F32, tag="touch")
    for i, dma in enumerate(wk_dmas + [wv_dma]):
        t = nc.vector.memset(touch_sb[0:1, i : i + 1], 0.0)
        tile.add_dep_helper(t.ins, dma.ins, reason="observe wkv DMA on DVE")

    # Phase 2a: qT[e, i] for ALL (permuted) queries of the batch
    for e in range(EC):
        for it in range(NQ // F):
            ps = psmain.tile([P, F], F32, tag="ps")
            for c in range(DC):
                nc.tensor.matmul(
                    ps,
                    lhsT=wq_sb[:, c, e * P : (e + 1) * P],
                    rhs=xT_sb[:, c, it * F : (it + 1) * F],
                    start=(c == 0),
                    stop=(c == DC - 1),
                )
            nc.vector.tensor_copy(out=qT_sb[:, e, it * F : (it + 1) * F], in_=ps)

    # Phase 2b: scoresT[j, i] = k @ q.T over this key half, p = exp(s*SCALE)
    for j in range(JB):
        for it in range(NQ // F):
            ps = psmain.tile([P, F], F32, tag="ps")
            for e in range(EC):
                nc.tensor.matmul(
                    ps,
                    lhsT=kT_sb[:, e, j * P : (j + 1) * P],
                    rhs=qT_sb[:, e, it * F : (it + 1) * F],
                    start=(e == 0),
                    stop=(e == EC - 1),
                )
            last_exp = nc.scalar.activation(
                out=pT_sb[:, j, it * F : (it + 1) * F],
                in_=ps,
                func=mybir.ActivationFunctionType.Exp,
                scale=float(SCALE),
            )

    for d in in_dmas:
        sp_observe(d, "observe input DMA on SP")

    # Phase 2c: partial out[i, 0:1024] = pT.T @ v, partial denom in column
    # 1024 (folded into the same output tensor so there are exactly 8
    # stores — one lap of the 8 SWDGE queues; a second lap would add a
    # queue-order wait on top of the data-ready wait).  Stored bf16.
    outr = out.rearrange("(g two p) e -> g p two e", two=2, p=P)
    oguard = small.tile([P, NQ // (2 * P)], F32, tag="oguard")
    out_dmas = []
    for ib2 in range(NQ // (2 * P)):
        o_sb = outp.tile([P, 2, D + 1], BF, tag="o")
        g = None
        if ib2 >= 2:
            # Pre-observe the output-DMA tick (WAR on o_sb slot reuse) on
            # the DVE so the copies below carry only their one data wait.
            g = nc.vector.memset(oguard[0:1, ib2 : ib2 + 1], 0.0)
            tile.add_dep_helper(
                g.ins, out_dmas[ib2 - 2].ins, reason="observe out DMA on DVE"
            )
        # Absorb the WAW against the slot's previous DVE writes in a guard
        # write of its own, so the data copies keep a single wait each.
        g2 = nc.vector.memset(o_sb[0:1, 0, 0:1], 0.0)
        if g is not None:
            tile.add_dep_helper(g2.ins, g.ins, False, reason="order after oguard")
        for t in range(2):
            ib = 2 * ib2 + t
            po0 = psav.tile([P, F], F32, tag="po")
            po1 = psav.tile([P, F], F32, tag="po")
            pd = psav.tile([P, F], F32, tag="po")
            for j in range(JB):
                lhsT = pT_sb[:, j, ib * P : (ib + 1) * P]
                nc.tensor.matmul(
                    po0, lhsT=lhsT, rhs=v_sb[:, j, 0:F],
                    start=(j == 0), stop=(j == JB - 1),
                )
                nc.tensor.matmul(
                    po1, lhsT=lhsT, rhs=v_sb[:, j, F : 2 * F],
                    start=(j == 0), stop=(j == JB - 1),
                )
                last_mm = nc.tensor.matmul(
                    pd[:, 0:1], lhsT=lhsT, rhs=ones_sb,
                    start=(j == 0), stop=(j == JB - 1),
                )
            # Denominator copy first: pd's stop-matmul is the group's last
            # PE tick, so this copy's PE wait covers po0/po1 and the po
            # copies need only their (buffer-reuse) DVE wait.  The explicit
            # sync=False deps pin the scheduler to that order.
            dcp = nc.vector.tensor_copy(
                out=o_sb[:, t, D : D + 1], in_=pd[:, 0:1]
            )
            tile.add_dep_helper(dcp.ins, g2.ins, False, reason="order after guard")
            c0 = nc.vector.tensor_copy(out=o_sb[:, t, 0:F], in_=po0)
            tile.add_dep_helper(c0.ins, dcp.ins, False, reason="order after dcp")
            last_cp = nc.vector.tensor_copy(out=o_sb[:, t, F : 2 * F], in_=po1)
            tile.add_dep_helper(last_cp.ins, c0.ins, False, reason="order after c0")
        out_dmas.append(nc.gpsimd.dma_start(out=outr[ib2], in_=o_sb))

    # Let SP observe every remaining proc's final tick so the auto-generated
    # kernel-tail drain needs no aggregated multi-sem wait of its own.
    for dd in out_dmas:
        sp_observe(dd, "observe output DMA on SP")
    sp_observe(last_exp, "observe ACT on SP")
    sp_observe(last_mm, "observe PE on SP")
    sp_observe(last_cp, "observe DVE on SP")


def build_attention_module():
    nc = bass.Bass(trn_type="TRN2", target_bir_lowering=False, debug=False)
    xT = nc.dram_tensor("xT", [D, NQ], BF, kind="ExternalInput").ap()
    wqT = nc.dram_tensor("wqT", [D, D], BF, kind="ExternalInput").ap()
    wkT = nc.dram_tensor("wkT", [D, D], BF, kind="ExternalInput").ap()
    wvT = nc.dram_tensor("wvT", [D, D], BF, kind="ExternalInput").ap()
    out = nc.dram_tensor("out", [NQ, D + 1], BF, kind="ExternalOutput").ap()
    with tile.TileContext(nc) as tc:
        with ExitStack() as ctx:
            _attention_kernel(ctx, tc, out, xT, wqT, wkT, wvT)
    return nc


_module_cache = None


def _get_module():
    global _module_cache
    if _module_cache is None:
        _module_cache = build_attention_module()
    return _module_cache


def make_in_maps(x, Wq, Wk, Wv):
    bf = ml_dtypes.bfloat16
    x = np.asarray(x, dtype=np.float32)
    wq = np.asarray(Wq, dtype=np.float32).T.astype(bf)
    wk = np.asarray(Wk, dtype=np.float32).T.astype(bf)
    wv = np.asarray(Wv, dtype=np.float32).T.astype(bf)
    in_maps = []
    for core in range(NCORES):
        b, half = divmod(core, 2)
        xt = x[b].T  # [D, N]
        if half:
            xt = np.concatenate([xt[:, NKH:], xt[:, :NKH]], axis=1)
        in_maps.append(
            {
                "xT": np.ascontiguousarray(xt).astype(bf),
                "wqT": wq,
                "wkT": wk,
                "wvT": wv,
            }
        )
    return in_maps


def _install_ntff_hook_shim():
    """The container's `antenv` stub lacks axon_hooks; register an equivalent
    built on trn_agent_boot's ctypes NTFF driver so trace=True works."""
    import sys
    import types

    if "antenv.axon_hooks" in sys.modules:
        return
    try:
        from trn_agent_boot.trn_boot import _ntff_profile_via_ctypes

        hook = _ntff_profile_via_ctypes("/opt/axon/libaxon_pjrt.so")
    except Exception:
        hook = None
    mod = types.ModuleType("antenv.axon_hooks")
    mod.get_axon_ntff_profile_hook = lambda: hook
    sys.modules["antenv.axon_hooks"] = mod


def kernel(x, Wq, Wk, Wv, _trace=False, _trace_cores=None):
    if _trace:
        _install_ntff_hook_shim()
    in_maps = make_in_maps(x, Wq, Wk, Wv)
    nc = _get_module()
    res = run_bass_kernel_spmd(
        nc,
        in_maps,
        core_ids=list(range(NCORES)),
        trace=_trace,
        trace_cores=_trace_cores,
    )
    out = np.empty((B, N, D), dtype=np.float32)
    for b in range(B):
        r0 = res.results[2 * b]["out"].astype(np.float32)
        r1 = res.results[2 * b + 1]["out"].astype(np.float32)
        # core 2b+1's query rows are permuted (its key half first): un-permute.
        r1 = np.concatenate([r1[NKH:], r1[:NKH]], axis=0)
        osum = r0 + r1
        out[b] = osum[:, :D] / osum[:, D : D + 1]
    if _trace:
        return out, res
    return out


# revision 42
# speedup vs baseline: 1.1715x; 1.1393x over previous
"""Single-head attention (B=4, N=2048, D=1024) on 8 Trainium2 NeuronCores.

Sharding: core c handles batch c//2 and KEY half c%2, with the duplicated
Q projection eliminated by a pairwise AllGather.  Each core receives only
its key-half of x (xTk, 2MB) plus the three weights; it computes K/V
projections and Q for its OWN 1024 queries (= its key half), then
AllGathers the pair's qT shards through DRAM bounce buffers while the
remaining K/V projection work hides the collective's ~27us fixed latency.
Scores/AV then run over its 1024 keys x all 2048 queries (global order),
producing the partial (unnormalized) attention output and partial softmax
denominator.  The host combines halves: out = (oA + oB) / (dA + dB).

Precision: projections/AV in bf16 (fp32 PSUM accumulation).  The scores
contraction is split: e-blocks 0-3 bf16, e-blocks 4-7 fp8e4 DoubleRow
(two contraction rows per PE cell, 2x bf16 throughput measured).  This
half-fp8 split measures rel err ~1.25e-2 against the f32 reference (full
fp8 was 1.89e-2 — too close to the 2e-2 gate).  exp in fp32 on the
scalar engine; unnormalized softmax (no max subtraction) is safe since
|scores/sqrt(D)| is ~N(0, 0.33^2).  Partial outputs are stored bf16.
"""

from contextlib import ExitStack

import ml_dtypes
import numpy as np

import concourse.bass as bass
import concourse.mybir as mybir
import concourse.tile as tile
from concourse.bass_utils import run_bass_kernel_spmd

B, N, D = 4, 2048, 1024
NCORES = 8
P = 128
NQ = N            # total queries per batch (gathered)
NKH = N // 2      # keys (and local queries) per core
DC = D // P       # 8 contraction chunks
EC = D // P       # 8 embed blocks
JB = NKH // P     # 8 key blocks
F = 512           # matmul moving free dim (one PSUM bank of fp32)
SCALE = 1.0 / np.sqrt(D)
N_WARM_PRE = 9    # back-to-back dummy matmuls (~3.8us cold) to warm HAM
NBF = 4           # scores e-blocks 0..NBF-1 in bf16; the rest fp8 DoubleRow

BF = mybir.dt.bfloat16
F8 = mybir.dt.float8e4
F32 = mybir.dt.float32

REPLICA_GROUPS = [[0, 1], [2, 3], [4, 5], [6, 7]]

QBF_B = NBF * NKH * 2                 # bytes of the bf16 qT/kT half: 8192
QRANK_B = QBF_B + (EC - NBF) * NKH    # bytes per rank shard: 12288


def _attention_kernel(ctx, tc, out, xTk, wqT, wkT, wvT):
    nc = tc.nc

    consts = ctx.enter_context(tc.tile_pool(name="consts", bufs=1))
    psmain = ctx.enter_context(tc.tile_pool(name="psmain", bufs=2, space="PSUM"))
    psav = ctx.enter_context(tc.tile_pool(name="psav", bufs=6, space="PSUM"))
    outp_big = ctx.enter_context(tc.tile_pool(name="outp_big", bufs=1))
    outp_sm = ctx.enter_context(tc.tile_pool(name="outp_sm", bufs=3))
    small = ctx.enter_context(tc.tile_pool(name="small", bufs=2))
    dram = ctx.enter_context(tc.tile_pool(name="dram", bufs=1, space="DRAM"))

    # Resident SBUF tensors.  qT/kT/qTloc are byte-granular tiles holding
    # a bf16 half (e-blocks 0-3) and an fp8 half (e-blocks 4-7) exposed
    # through bitcast views, so every gather hop is ONE DMA.  qT is
    # rank-major: rank r's shard is a contiguous per-partition byte range.
    xTk_sb = consts.tile([P, DC, NKH], BF, tag="xTk")    # [p, d-chunk, key]
    wkv_sb = consts.tile([P, 2 * DC * D], BF, tag="wkv")
    wk_sb = wkv_sb.rearrange("p (two c e) -> p two c e", two=2, c=DC)[:, 0]
    wv_sb = wkv_sb.rearrange("p (two c e) -> p two c e", two=2, c=DC)[:, 1]
    wq_sb = consts.tile([P, DC, D], BF, tag="wq")
    qT_sb = consts.tile([P, 2, QRANK_B], F8, tag="qT")
    qTloc_sb = consts.tile([P, QRANK_B], F8, tag="qTloc")
    kT_sb = consts.tile([P, QRANK_B], F8, tag="kT")
    v_sb = consts.tile([P, JB, D], BF, tag="v")          # [p, key-block, e]
    pT_sb = consts.tile([P, JB, NQ], BF, tag="pT")       # [p, key-block, query]
    ones_sb = consts.tile([P, 1], BF, tag="ones")

    def _views(t):  # byte range -> (bf16 [P,NBF,NKH], fp8 [P,EC-NBF,NKH])
        bf = t[:, 0:QBF_B].bitcast(BF).rearrange("p (e j) -> p e j", e=NBF)
        f8 = t[:, QBF_B:QRANK_B].rearrange("p (e j) -> p e j", e=EC - NBF)
        return bf, f8

    qTloc_bf, qTloc_f8 = _views(qTloc_sb)
    kT_bf, kT_f8 = _views(kT_sb)
    qT_rk = [_views(qT_sb[:, r, :]) for r in range(2)]

    # DRAM bounce buffers for the pairwise qT AllGather (mixed payload:
    # 1.5MB out, 3MB back).
    cc_in = dram.tile([P, QRANK_B], F8, name="cc_in")
    cc_out = dram.tile([2, P, QRANK_B], F8, name="cc_out")

    nc.vector.memset(ones_sb, 1.0)

    xTr = xTk.rearrange("(c p) j -> p c j", p=P)
    wqr = wqT.rearrange("(c p) e -> p c e", p=P)
    wkr = wkT.rearrange("(c p) e -> p c e", p=P)
    wvr = wvT.rearrange("(c p) e -> p c e", p=P)

    # Input DMAs.  The per-core HBM read port (~358 GB/s) is the early
    # bottleneck: 8MB of input takes ~22us to land.  Phase 1a needs
    # wk + xTk (4MB) chunk-by-chunk ASAP, then wq chunks feed phase
    # 2a-local; wv (needed last, ~60us) follows as one large DMA.
    # Per-queue FIFO on the HWDGE queues preserves this priority.
    xk_dmas = []
    wk_dmas = []
    for c in range(DC):
        xk_dmas.append(nc.sync.dma_start(out=xTk_sb[:, c, :], in_=xTr[:, c, :]))
        wk_dmas.append(nc.sync.dma_start(out=wk_sb[:, c, :], in_=wkr[:, c, :]))
    wq_dmas = []
    for c in range(DC):
        wq_dmas.append(nc.sync.dma_start(out=wq_sb[:, c, :], in_=wqr[:, c, :]))
    wv_dma = nc.sync.dma_start(out=wv_sb[:, :, :], in_=wvr)
    in_dmas = xk_dmas + wk_dmas + wq_dmas + [wv_dma]

    def sp_observe(inst, why):
        n = nc.sync.nop(hint="observe")
        tile.add_dep_helper(n.ins, inst.ins, reason=why)

    # One PSUM tile for dummy warm-up and touch matmuls.  It comes from the
    # psav pool, which no DVE copy reads until phase 2c — so every write to
    # it is PE-local and touch matmuls carry exactly one (DMA) wait.
    warm_src = small.tile([P, 640], BF, tag="warm")
    nc.vector.memset(warm_src, 0.0)
    warm_ps = psav.tile([P, F], F32, tag="po")

    def dummy():
        nc.tensor.matmul(
            warm_ps, lhsT=warm_src[:, 0:P], rhs=warm_src[:, P : P + F],
            start=True, stop=True,
        )

    def touch(t):
        # Trivial matmul whose only purpose is to make the PE observe t's
        # producer (single sync wait), so later real matmuls need none.
        nc.tensor.matmul(
            warm_ps[0:1, 0:1], lhsT=t[:, 0:1], rhs=t[:, 0:1], start=True, stop=True
        )

    # Solid warm-up block: HAM un-throttles only after a ~3.4us window of
    # SUSTAINED PE activity; scattered chunk-gated matmuls never produce one.
    for _ in range(N_WARM_PRE):
        dummy()

    def kT_out(e, jt):
        if e < NBF:
            return kT_bf[:, e, jt * F : (jt + 1) * F]
        return kT_f8[:, e - NBF, jt * F : (jt + 1) * F]

    # Phase 1a e=0: kT[0, j] — chunk-major, gated on each (xk, wk) chunk
    # pair as it lands (~1.4us apart at HBM rate), with touch matmuls
    # carrying the DMA waits and interleaved dummies keeping the PE duty
    # high so HAM stays warm.
    ps0 = psmain.tile([P, F], F32, tag="ps")
    ps1 = psmain.tile([P, F], F32, tag="ps")
    for c in range(DC):
        touch(xTk_sb[:, c, :])
        touch(wk_sb[:, c, :])
        nc.tensor.matmul(
            ps0, lhsT=wk_sb[:, c, 0:P], rhs=xTk_sb[:, c, 0:F],
            start=(c == 0), stop=(c == DC - 1),
        )
        nc.tensor.matmul(
            ps1, lhsT=wk_sb[:, c, 0:P], rhs=xTk_sb[:, c, F : 2 * F],
            start=(c == 0), stop=(c == DC - 1),
        )
        dummy()
        dummy()
    nc.vector.tensor_copy(out=kT_out(0, 0), in_=ps0)
    nc.vector.tensor_copy(out=kT_out(0, 1), in_=ps1)

    # Phase 2a-local RIGHT AFTER the gated block: qT[e, j_local] for this
    # core's OWN 1024 queries, so the AllGather chain launches ~50us in.
    # Its first groups gate on the wq chunk stream (landing ~20-26us);
    # interleaved dummies keep the duty high.  Only wq chunk 0 needs a
    # touch: each group's START matmul carries the PSUM-reuse wait, so it
    # must not also wait on a DMA; later chunks' waits ride legally on the
    # non-start matmuls (one wait each).
    touch(wq_sb[:, 0, :])
    for e in range(EC):
        for it in range(NKH // F):
            ps = psmain.tile([P, F], F32, tag="ps")
            for c in range(DC):
                nc.tensor.matmul(
                    ps,
                    lhsT=wq_sb[:, c, e * P : (e + 1) * P],
                    rhs=xTk_sb[:, c, it * F : (it + 1) * F],
                    start=(c == 0),
                    stop=(c == DC - 1),
                )
                if e == 0:
                    dummy()
            qdst = (
                qTloc_bf[:, e, it * F : (it + 1) * F]
                if e < NBF
                else qTloc_f8[:, e - NBF, it * F : (it + 1) * F]
            )
            nc.vector.tensor_copy(out=qdst, in_=ps)
    # Ship the local shard and AllGather across the core pair.  All bounce
    # traffic rides gpsimd's SWDGE queues (3 + 4 output stores = 7 DMAs,
    # at most one per queue — no queue-lap waits).  One DMA per hop so
    # each instruction carries a single sync wait (the collective cannot
    # aggregate multiple input-piece semaphores).
    cc_in_dma = nc.gpsimd.dma_start(out=cc_in[:, :], in_=qTloc_sb[:, :])
    cc = nc.gpsimd.collective_compute(
        "AllGather",
        mybir.AluOpType.bypass,
        replica_groups=REPLICA_GROUPS,
        ins=[cc_in[:, :].opt()],
        outs=[cc_out[:, :, :].opt()],
    )

    # Phase 1a remainder (kT e-blocks 1-7) + wv touch (wv lands ~31us; the
    # PE reaches e=4 ~62us).
    def kT_block(e):
        for jt in range(NKH // F):
            ps = psmain.tile([P, F], F32, tag="ps")
            for c in range(DC):
                nc.tensor.matmul(
                    ps,
                    lhsT=wk_sb[:, c, e * P : (e + 1) * P],
                    rhs=xTk_sb[:, c, jt * F : (jt + 1) * F],
                    start=(c == 0),
                    stop=(c == DC - 1),
                )
            nc.vector.tensor_copy(out=kT_out(e, jt), in_=ps)

    for e in range(1, EC):
        kT_block(e)
        if e == 4:
            touch(wv_sb[:, 0, :])

    # Phase 1b: v[j, e] — lhsT = xTk[d, j-blk], rhs = WvT[d, e-tile]
    for j in range(JB):
        for et in range(D // F):
            ps = psmain.tile([P, F], F32, tag="ps")
            for c in range(DC):
                nc.tensor.matmul(
                    ps,
                    lhsT=xTk_sb[:, c, j * P : (j + 1) * P],
                    rhs=wv_sb[:, c, et * F : (et + 1) * F],
                    start=(c == 0),
                    stop=(c == DC - 1),
                )
            nc.vector.tensor_copy(out=v_sb[:, j, et * F : (et + 1) * F], in_=ps)

    # Read the gathered qT shards back, one DMA per rank, STAGGERED (the
    # nop serializes rank 1 behind rank 0) so rank 0 gets full read
    # bandwidth and phase 2b can start on its query tiles ~2us sooner.
    # qT_sb is fresh, so each read-back's only dependency is the previous
    # hop — exactly one wait per DMA.
    qt_rb0 = nc.gpsimd.dma_start(out=qT_sb[:, 0, :], in_=cc_out[0])
    n_rb = nc.gpsimd.nop(hint="observe")
    tile.add_dep_helper(n_rb.ins, qt_rb0.ins, reason="stagger rank-1 read-back")
    qt_rb1 = nc.gpsimd.dma_start(out=qT_sb[:, 1, :], in_=cc_out[1])

    # Phase 2b: scoresT[j, i] = k @ q.T over this key half, p = exp(s*SCALE).
    # Mixed contraction: e-blocks 0-3 bf16, e-blocks 4-7 as two fp8
    # DoubleRow matmuls (3D APs [128, 2, n]; middle dim = the interleaved
    # contraction-row pair), accumulating into one PSUM group.  Tiles are
    # processed rank-major (all rank-0 query tiles first) so compute can
    # begin as soon as rank 0's read-back lands; within a rank the
    # bf16/DoubleRow order snakes so consecutive tiles share the PE
    # weight-path mode at the boundary (mode switches cost ~200ns).
    def scores_tile(j, rk, itr, flip):
        q_bf, q_f8 = qT_rk[rk]
        ps = psmain.tile([P, F], F32, tag="ps")
        bf_mms = [
            dict(
                lhsT=kT_bf[:, e, j * P : (j + 1) * P],
                rhs=q_bf[:, e, itr * F : (itr + 1) * F],
                perf_mode=None,
            )
            for e in range(NBF)
        ]
        f8_mms = [
            dict(
                lhsT=kT_f8[:, e : e + 2, j * P : (j + 1) * P],
                rhs=q_f8[:, e : e + 2, itr * F : (itr + 1) * F],
                perf_mode=mybir.MatmulPerfMode.DoubleRow,
            )
            for e in range(0, EC - NBF, 2)
        ]
        mms = bf_mms + f8_mms if not flip else f8_mms + bf_mms
        for i, kw in enumerate(mms):
            nc.tensor.matmul(
                ps,
                lhsT=kw["lhsT"],
                rhs=kw["rhs"],
                start=(i == 0),
                stop=(i == len(mms) - 1),
                perf_mode=kw["perf_mode"],
            )
        return ps

    flip = False
    last_exp = None
    for rk in range(2):
        # Absorb this rank's read-back DMA wait on the PE.
        touch(qT_sb[:, rk, 0:1])
        for j in range(JB):
            for itr in range(NKH // F):
                it = rk * (NKH // F) + itr
                ps = scores_tile(j, rk, itr, flip)
                flip = not flip
                last_exp = nc.scalar.activation(
                    out=pT_sb[:, j, it * F : (it + 1) * F],
                    in_=ps,
                    func=mybir.ActivationFunctionType.Exp,
                    scale=float(SCALE),
                )

    for dmad in in_dmas:
        sp_observe(dmad, "observe input DMA on SP")
    sp_observe(cc_in_dma, "observe cc bounce-in DMA on SP")

    # Phase 2c: partial out[i, 0:1024] = pT.T @ v, partial denom in column
    # 1024 (folded into the same output tensor).  FOUR stores sized
    # {5,1,1,1} query-block groups: the big store issues mid-phase when
    # write bandwidth is free, the three small ones trickle out ~3.6us
    # apart, so the end-of-kernel drain only covers 525KB.  Every store
    # has its own buffer — no WAR guards needed.  Stored bf16.
    outr = out.rearrange("(gg p) e -> p gg e", p=P)   # [P, 16, D+1]
    STORES = [(0, 5), (5, 1), (6, 1), (7, 1)]         # (start ib2, n ib2)
    out_dmas = []
    for s, (start, ng) in enumerate(STORES):
        pool = outp_big if ng > 1 else outp_sm
        o_sb = pool.tile([P, 2 * ng, D + 1], BF, tag="o")
        g2 = nc.vector.memset(o_sb[0:1, 0, 0:1], 0.0)
        for gi in range(ng):
            ib2 = start + gi
            for t in range(2):
                ib = 2 * ib2 + t
                tl = 2 * gi + t
                po0 = psav.tile([P, F], F32, tag="po")
                po1 = psav.tile([P, F], F32, tag="po")
                pd = psav.tile([P, F], F32, tag="po")
                for j in range(JB):
                    lhsT = pT_sb[:, j, ib * P : (ib + 1) * P]
                    nc.tensor.matmul(
                        po0, lhsT=lhsT, rhs=v_sb[:, j, 0:F],
                        start=(j == 0), stop=(j == JB - 1),
                    )
                    nc.tensor.matmul(
                        po1, lhsT=lhsT, rhs=v_sb[:, j, F : 2 * F],
                        start=(j == 0), stop=(j == JB - 1),
                    )
                    last_mm = nc.tensor.matmul(
                        pd[:, 0:1], lhsT=lhsT, rhs=ones_sb,
                        start=(j == 0), stop=(j == JB - 1),
                    )
                # Denominator copy first: pd's stop-matmul is the group's
                # last PE tick, so this copy's PE wait covers po0/po1 and
                # the po copies need only their (buffer-reuse) DVE wait.
                dcp = nc.vector.tensor_copy(
                    out=o_sb[:, tl, D : D + 1], in_=pd[:, 0:1]
                )
                tile.add_dep_helper(
                    dcp.ins, g2.ins, False, reason="order after guard"
                )
                c0 = nc.vector.tensor_copy(out=o_sb[:, tl, 0:F], in_=po0)
                tile.add_dep_helper(c0.ins, dcp.ins, False, reason="order after dcp")
                last_cp = nc.vector.tensor_copy(out=o_sb[:, tl, F : 2 * F], in_=po1)
                tile.add_dep_helper(last_cp.ins, c0.ins, False, reason="order after c0")
        out_dmas.append(
            nc.gpsimd.dma_start(
                out=outr[:, 2 * start : 2 * (start + ng), :], in_=o_sb
            )
        )

    for dd in out_dmas:
        sp_observe(dd, "observe output DMA on SP")
    sp_observe(qt_rb0, "observe qT read-back 0 on SP")
    sp_observe(qt_rb1, "observe qT read-back 1 on SP")
    sp_observe(last_exp, "observe ACT on SP")
    sp_observe(last_mm, "observe PE on SP")
    sp_observe(last_cp, "observe DVE on SP")


def build_attention_module():
    nc = bass.Bass(trn_type="TRN2", target_bir_lowering=False, debug=False)
    xTk = nc.dram_tensor("xTk", [D, NKH], BF, kind="ExternalInput").ap()
    wqT = nc.dram_tensor("wqT", [D, D], BF, kind="ExternalInput").ap()
    wkT = nc.dram_tensor("wkT", [D, D], BF, kind="ExternalInput").ap()
    wvT = nc.dram_tensor("wvT", [D, D], BF, kind="ExternalInput").ap()
    out = nc.dram_tensor("out", [NQ, D + 1], BF, kind="ExternalOutput").ap()
    with tile.TileContext(nc) as tc:
        with ExitStack() as ctx:
            _attention_kernel(ctx, tc, out, xTk, wqT, wkT, wvT)
    return nc


_module_cache = None


def _get_module():
    global _module_cache
    if _module_cache is None:
        _module_cache = build_attention_module()
    return _module_cache


def make_in_maps(x, Wq, Wk, Wv):
    bf = ml_dtypes.bfloat16
    x = np.asarray(x, dtype=np.float32)
    wq = np.asarray(Wq, dtype=np.float32).T.astype(bf)
    wk = np.asarray(Wk, dtype=np.float32).T.astype(bf)
    wv = np.asarray(Wv, dtype=np.float32).T.astype(bf)
    in_maps = []
    for core in range(NCORES):
        b, half = divmod(core, 2)
        xtk = x[b].T[:, half * NKH : (half + 1) * NKH]  # [D, NKH]
        in_maps.append(
            {
                "xTk": np.ascontiguousarray(xtk).astype(bf),
                "wqT": wq,
                "wkT": wk,
                "wvT": wv,
            }
        )
    return in_maps


def _install_ntff_hook_shim():
    """The container's `antenv` stub lacks axon_hooks; register an equivalent
    built on trn_agent_boot's ctypes NTFF driver so trace=True works."""
    import sys
    import types

    if "antenv.axon_hooks" in sys.modules:
        return
    try:
        from trn_agent_boot.trn_boot import _ntff_profile_via_ctypes

        hook = _ntff_profile_via_ctypes("/opt/axon/libaxon_pjrt.so")
    except Exception:
        hook = None
    mod = types.ModuleType("antenv.axon_hooks")
    mod.get_axon_ntff_profile_hook = lambda: hook
    sys.modules["antenv.axon_hooks"] = mod


def kernel(x, Wq, Wk, Wv, _trace=False, _trace_cores=None):
    if _trace:
        _install_ntff_hook_shim()
    in_maps = make_in_maps(x, Wq, Wk, Wv)
    nc = _get_module()
    res = run_bass_kernel_spmd(
        nc,
        in_maps,
        core_ids=list(range(NCORES)),
        trace=_trace,
        trace_cores=_trace_cores,
    )
    out = np.empty((B, N, D), dtype=np.float32)
    for b in range(B):
        r0 = res.results[2 * b]["out"].astype(np.float32)
        r1 = res.results[2 * b + 1]["out"].astype(np.float32)
        osum = r0 + r1
        out[b] = osum[:, :D] / osum[:, D : D + 1]
    if _trace:
        return out, res
    return out
